# revision 41
# baseline (speedup 1.0000x reference)
"""Trainium2 Bass kernel for CustomMHA (B=4, S=2048, D=1024, H=16).

Sharding: 8 cores = 4 batches x 2 head-groups. Core c handles batch c//2,
heads (c%2)*8 .. (c%2)*8+7. Each core computes its heads' QKV projection,
attention, and a partial output projection (its heads' columns of W_o);
the host sums the two partial Y's per batch.

Per-core structure (bf16 matmuls, fp32 PSUM accumulation):
  - x^T [1024, 2048] resident in SBUF; Q^T/K^T per head-pair as
    [dout, token] tiles (two heads on partition halves 0-63 / 64-127),
    V as [token, head, dh+1] with a ones column for the denominator.
  - scores S^T[k, q] per 128-k tile; the two heads of a pair are packed
    into PE row groups (dh=64 contraction at partition base 0 and 64)
    writing the two halves of one [128, 1024] PSUM tile.
  - softmax: exp on ScalarE with 1/sqrt(d_h) folded into the activation
    scale; no max-subtraction (|scores|/8 stays < ~5).
  - AV: lhsT = [V_h | 1] (M=65), so PSUM row 64 accumulates the softmax
    denominator for free. AV matmuls trail the exp by 2 k-tiles so their
    LDWEIGHTS is never gated on the exp semaphore.
  - normalization: the [65,512] PSUM->SBUF copies run on ScalarE (off the
    DVE FIFO, so the next qb's AV-start isn't gated behind the chain),
    then DVE den-row bounce to partition 0 + reciprocal_approx_fast
    (the custom DVE op needs partition-aligned in/out) + gpsimd
    partition_broadcast + DVE multiply. Head b bounces through a
    [64,512] tile + SBUF->SBUF DMA to reach partitions 64-127.
  - projection: Y[token, e] accumulated over the 4 pair-chunks.
Schedule: x streams in two 1024-column DMA waves; the prologue emits
K0/Q0-tb0/V0 per wave so the PE starts ~6us after the preamble. The
rest of Q0, pairs 1-3 V (N=384 pass) and each next pair's QK interleave
into the exp-bound attention ktile loop (consume 2-3 generator steps
per ktile). Output projection for q-block g interleaves into pair-3's
qb g+1; 28 scratch matmuls keep the PE HAM clock-gate warm through the
final norm chain so the tail projection runs at 2.4GHz.

fp8 (DoubleRow) variants for AV/QKV/proj were numerically simulated and
rejected: e4m3 rounding of pt/V alone gives rel_err ~0.023 > the 2e-2
gate (see precision_sim2.py). The kernel is PE-bound at bf16's
1 column/cycle; measured PE busy ~350us of ~396us span.
"""

import os
import numpy as np
import ml_dtypes

B, S, D, H, DH = 4, 2048, 1024, 16, 64
NCORES = 8
P = 128

_cache = {}


def _build():
    import concourse.bacc as bacc
    import concourse.tile as tile
    from concourse import mybir

    f32 = mybir.dt.float32
    bf16 = mybir.dt.bfloat16
    Exp = mybir.ActivationFunctionType.Exp

    nc = bacc.Bacc("TRN2", target_bir_lowering=False, debug=False)
    xT = nc.dram_tensor("xT", [P, 8, S], bf16, kind="ExternalInput")
    # wqkp: [d, pair, 256] pair-major (cols 0-127 Q-dout, 128-255 K-dout)
    wqkp = nc.dram_tensor("wqkp", [P, 8, 4, 256], bf16, kind="ExternalInput")
    wv = nc.dram_tensor("wv", [P, 8, 512], bf16, kind="ExternalInput")
    wo = nc.dram_tensor("wo", [P, 4, D], bf16, kind="ExternalInput")
    y = nc.dram_tensor("y", [S, D], f32, kind="ExternalOutput")

    with tile.TileContext(nc) as tc:
        import contextlib
        stack = contextlib.ExitStack()
        with stack:
            sb = stack.enter_context(tc.tile_pool(name="sb", bufs=1))
            ptp = stack.enter_context(tc.tile_pool(name="ptp", bufs=18))
            nrm = stack.enter_context(tc.tile_pool(name="nrm", bufs=2))
            otbp = stack.enter_context(tc.tile_pool(name="otb", bufs=4))
            yp = stack.enter_context(tc.tile_pool(name="yp", bufs=2))
            # PSUM: scores 2x[128,1024] (8KB) + AV 2x[65,512] (4KB) +
            # qkv 2x[128,512] (4KB, reused by proj after close) = 16KB
            psS = stack.enter_context(tc.tile_pool(name="psS", bufs=2, space="PSUM"))
            psO = stack.enter_context(tc.tile_pool(name="psO", bufs=1, space="PSUM"))

            qts = [sb.tile([P, S], bf16, tag=f"qt{p}", name=f"qt{p}") for p in range(4)]
            kts = [sb.tile([P, S], bf16, tag=f"kt{p}", name=f"kt{p}") for p in range(4)]
            ots = [[sb.tile([P, 512], bf16, tag=f"ot{p}_{q}", name=f"ot{p}_{q}")
                    for q in range(4)] for p in range(4)]
            vts = [sb.tile([P, 16, 2, 65], bf16, tag=f"vt{p}", name=f"vt{p}") for p in range(4)]
            wo_sb = sb.tile([P, 4, D], bf16)
            x_sbs = [sb.tile([P, S], bf16, tag=f"x{c}", name=f"x{c}")
                     for c in range(8)]
            wqk_sbs = [sb.tile([P, 8, 256], bf16, tag=f"wqk{j}", name=f"wqk{j}")
                       for j in range(4)]
            wv_sb = sb.tile([P, 8, 512], bf16)
            ones64 = sb.tile([1, 64], f32)

            # input DMAs (all partition-major contiguous). x streams in two
            # 1024-column waves (2KB per partition row keeps DMA descriptors
            # efficient) so the prologue's K0/Q0/V0 matmuls start after the
            # first wave instead of waiting for the full 4MB.
            nc.sync.dma_start(out=wqk_sbs[0][:, 0:4, :], in_=wqkp[:, 0:4, 0, :])
            nc.sync.dma_start(out=wqk_sbs[0][:, 4:8, :], in_=wqkp[:, 4:8, 0, :])
            for w in range(2):
                for c in range(8):
                    if w == 0 and c < 2:
                        # first chunks split across partition halves (two DMA
                        # queues) so the prologue's first matmuls start sooner
                        for h in (0, 1):
                            nc.sync.dma_start(
                                out=x_sbs[c][64 * h:64 * (h + 1), 0:1024],
                                in_=xT[64 * h:64 * (h + 1), c, 0:1024])
                    else:
                        nc.sync.dma_start(
                            out=x_sbs[c][:, w * 1024:(w + 1) * 1024],
                            in_=xT[:, c, w * 1024:(w + 1) * 1024])
                if w == 0:
                    nc.sync.dma_start(out=wv_sb[:], in_=wv[:])
            for j in range(1, 4):
                nc.sync.dma_start(out=wqk_sbs[j][:], in_=wqkp[:, :, j, :])
            nc.sync.dma_start(out=wo_sb[:], in_=wo[:])
            for p in range(4):
                nc.vector.memset(vts[p][:, :, :, 64:65], 1.0)
            nc.vector.memset(ones64[:], 1.0)

            def gen_qk(hp, pool, halves=(0, 1), tbs=(0, 1, 2, 3)):
                for half in halves:
                    dst = qts[hp] if half == 0 else kts[hp]
                    for tb in tbs:
                        ps = pool.tile([P, 512], f32, tag="ps", name="ps")
                        for c in range(8):
                            nc.tensor.matmul(
                                ps[:],
                                lhsT=wqk_sbs[hp][:, c, half * 128:(half + 1) * 128],
                                rhs=x_sbs[c][:, tb * 512:(tb + 1) * 512],
                                start=(c == 0), stop=(c == 7),
                            )
                            if c == 7:
                                nc.vector.tensor_copy(
                                    dst[:, tb * 512:(tb + 1) * 512], ps[:])
                            yield

            def gen_v(pool, pairs, ts=tuple(range(16))):
                # V projection for a contiguous run of head-pairs. Pair 0 runs
                # serially in the prologue (qb0's AV needs all 16 k-chunks);
                # pairs 1-3 interleave into pair-0/1 attention.
                lo, hi = pairs[0] * 128, (pairs[-1] + 1) * 128
                for t in ts:
                    ps = pool.tile([P, 512], f32, tag="ps", name="ps")
                    for c in range(8):
                        nc.tensor.matmul(
                            ps[:, 0:hi - lo],
                            lhsT=x_sbs[c][:, t * 128:(t + 1) * 128],
                            rhs=wv_sb[:, c, lo:hi],
                            start=(c == 0), stop=(c == 7),
                        )
                        if c == 7:
                            for k in pairs:
                                nc.vector.tensor_copy(
                                    vts[k][:, t, :, 0:64],
                                    ps[:, k * 128 - lo:(k + 1) * 128 - lo].rearrange(
                                        "p (h d) -> p h d", d=64))
                        yield

            def gen_proj_qb(g, pool):
                # projection for token tiles of q-block g (needs all ots[*][g]).
                # The no-op prefix delays the first matmul past the norm+bounce
                # chain that produces ots[3][g].
                for _ in range(16):
                    yield
                for tq in range(4):
                    t = g * 4 + tq
                    for eh in range(2):
                        if g == 3 and tq >= 2:
                            # tail groups: borrow the (now idle) scores pool
                            # so 4 psum buffers rotate and the drain isn't
                            # gated on each ysb copy freeing a psC bank
                            ps = psS.tile([P, 1024], f32, tag="s",
                                          name="ps")[:, 0:512]
                        else:
                            ps = pool.tile([P, 512], f32, tag="psy",
                                           name="ps")[:]
                        for c in range(4):
                            nc.tensor.matmul(
                                ps,
                                lhsT=ots[c][g][:, tq * 128:(tq + 1) * 128],
                                rhs=wo_sb[:, c, eh * 512:(eh + 1) * 512],
                                start=(c == 0), stop=(c == 3),
                            )
                            if c == 3:
                                ysb = yp.tile([P, 512], f32, tag="ysb", name="ysb")
                                nc.vector.tensor_copy(ysb[:], ps)
                                nc.sync.dma_start(
                                    out=y[t * 128:(t + 1) * 128,
                                          eh * 512:(eh + 1) * 512],
                                    in_=ysb[:])
                            yield

            work = []

            def consume(n):
                for _ in range(n):
                    while work:
                        try:
                            next(work[0])
                            break
                        except StopIteration:
                            work.pop(0)
                    else:
                        break

            def drain_work():
                while work:
                    for _ in work.pop(0):
                        pass

            def emit_attn(hp, after_qb=None, rate0=3):
                qt, kt, vt = qts[hp], kts[hp], vts[hp]
                for qb in range(4):
                    qsl = slice(qb * 512, (qb + 1) * 512)
                    ot = ots[hp][qb]
                    oa = psO.tile([65, 512], f32, tag="oa")
                    ob = psO.tile([65, 512], f32, tag="ob")
                    pts = [None] * 16

                    def emit_av(kti):
                        # on the stop tile, finish ob first: the osb copy
                        # (head b, which gates the next qb's AV-start and
                        # carries the bounce DMA) can then begin ~400ns sooner
                        heads = (1, 0) if kti == 15 else (0, 1)
                        for h in heads:
                            nc.tensor.matmul(
                                (oa, ob)[h][:], lhsT=vt[:, kti, h, :],
                                rhs=pts[kti][:, h * 512:(h + 1) * 512],
                                start=(kti == 0), stop=(kti == 15))

                    for kti in range(16):
                        ksl = slice(kti * 128, (kti + 1) * 128)
                        if kti >= 2:
                            emit_av(kti - 2)
                        s = psS.tile([P, 1024], f32, tag="s")
                        nc.tensor.matmul(
                            s[:, 0:512], lhsT=kt[0:64, ksl], rhs=qt[0:64, qsl],
                            start=True, stop=True)
                        nc.tensor.matmul(
                            s[:, 512:1024], lhsT=kt[64:128, ksl], rhs=qt[64:128, qsl],
                            start=True, stop=True)
                        pt = ptp.tile([P, 1024], bf16, tag="pt")
                        pts[kti] = pt
                        nc.scalar.activation(pt[:], s[:], Exp, scale=0.125)
                        consume(rate0 if qb == 0 else 3)
                    for kti in (14, 15):
                        emit_av(kti)
                    # free the AV psum banks fast: copy to SBUF on ScalarE
                    # (keeps the copies off the DVE FIFO so the next qb's
                    # AV-start isn't gated on the norm chain), then normalize.
                    # Head b first: its path carries the extra SBUF->SBUF
                    # bounce DMA, so it gates downstream readers.
                    osb = nrm.tile([65, 512], f32, tag="osb")
                    nc.scalar.copy(osb[:], ob[:])
                    osa = nrm.tile([65, 512], f32, tag="osa")
                    nc.scalar.copy(osa[:], oa[:])
                    # reciprocal_approx_fast needs partition-aligned in/out:
                    # bounce the denominator row from partition 64 to 0 first.
                    dnb = nrm.tile([1, 512], f32, tag="dnb")
                    nc.vector.tensor_copy(dnb[:], osb[64:65, :])
                    rcb = nrm.tile([1, 512], f32, tag="rcb")
                    nc.vector.reciprocal_approx_fast(rcb[:], dnb[:])
                    final = hp == 3 and qb == 3
                    if final:
                        # last norm gates the tail projection: broadcast via a
                        # PE ones-matmul into a free scores bank (~0.4us)
                        # instead of the 1.4us gpsimd partition_broadcast
                        bps = psS.tile([P, 1024], f32, tag="s", name="bps")
                        bcb = bps[0:64, 0:512]
                        nc.tensor.matmul(bcb, lhsT=ones64[:], rhs=rcb[:],
                                         start=True, stop=True)
                    else:
                        bcb = nrm.tile([64, 512], f32, tag="bcb", name="bcb")[:]
                        nc.gpsimd.partition_broadcast(bcb, rcb[:])
                    otb = otbp.tile([64, 512], bf16, tag="otb")
                    nc.vector.tensor_mul(otb[:], osb[0:64, :], bcb)
                    nc.sync.dma_start(out=ot[64:128, :], in_=otb[:])
                    # head a (lanes aligned 0-63)
                    dna = nrm.tile([1, 512], f32, tag="dna")
                    nc.vector.tensor_copy(dna[:], osa[64:65, :])
                    rca = nrm.tile([1, 512], f32, tag="rca")
                    nc.vector.reciprocal_approx_fast(rca[:], dna[:])
                    if final:
                        bps2 = psS.tile([P, 1024], f32, tag="s", name="bps2")
                        bca = bps2[0:64, 0:512]
                        nc.tensor.matmul(bca, lhsT=ones64[:], rhs=rca[:],
                                         start=True, stop=True)
                    else:
                        bca = nrm.tile([64, 512], f32, tag="bca", name="bca")[:]
                        nc.gpsimd.partition_broadcast(bca, rca[:])
                    nc.vector.tensor_mul(ot[0:64, :], osa[0:64, :], bca)
                    if after_qb is not None:
                        after_qb(qb)

            with tc.tile_pool(name="psA", bufs=2, space="PSUM") as psA:
                # serial prologue, paced by the x DMA waves: K0 + Q0-tb0 +
                # pair-0 V, each wave's work emitted as its x columns land.
                # The rest of Q0, pairs 1-3 V, and pair-1 QK interleave into
                # attention.
                for w in range(2):
                    for _ in gen_qk(0, psA, halves=(1,), tbs=(2 * w, 2 * w + 1)):
                        pass
                    if w == 0:
                        for _ in gen_qk(0, psA, halves=(0,), tbs=(0,)):
                            pass
                    for _ in gen_v(psA, (0,), ts=tuple(range(8 * w, 8 * w + 8))):
                        pass
                work[:] = [gen_qk(0, psA, halves=(0,), tbs=(1, 2, 3)),
                           gen_qk(1, psA),
                           gen_v(psA, (1, 2, 3))]
                emit_attn(0, rate0=2)
                work.append(gen_qk(2, psA))
                emit_attn(1)
                drain_work()
                work[:] = [gen_qk(3, psA)]
                emit_attn(2)
                drain_work()

            # ---- pair 3 attention + interleaved output projection ----
            with tc.tile_pool(name="psC", bufs=2, space="PSUM") as psC:
                def after3(qb):
                    work.append(gen_proj_qb(qb, psC))
                    if qb == 3:
                        # scratch matmuls keep the PE (HAM clock gate) warm
                        # through the final norm chain so the tail projection
                        # runs at 2.4GHz instead of re-throttled 1.2GHz
                        for _ in range(18):
                            sw = psS.tile([P, 1024], f32, tag="s", name="sw")
                            nc.tensor.matmul(
                                sw[:, 0:512], lhsT=kts[3][0:64, 0:128],
                                rhs=qts[3][0:64, 0:512], start=True, stop=True)

                emit_attn(3, after_qb=after3)
                drain_work()

    nc.compile()
    return nc


def _get_nc():
    if "nc" not in _cache:
        _cache["nc"] = _build()
    return _cache["nc"]


def make_in_maps(x, W_qkv, W_o):
    # All tensors pre-arranged partition-major [128, ...] so each input DMA
    # is 128 large contiguous descriptors.
    bf = ml_dtypes.bfloat16
    in_maps = []
    for c in range(NCORES):
        b, g = c // 2, c % 2
        ds = g * 512  # this core's slice of the head-major model dim
        # x^T [d, s] -> [p, c, s]
        xTc = np.ascontiguousarray(
            x[b].T.reshape(8, P, S).transpose(1, 0, 2).astype(bf))
        wq = W_qkv[ds:ds + 512, :].reshape(4, P, D)
        wk = W_qkv[1024 + ds:1024 + ds + 512, :].reshape(4, P, D)
        wqkc = np.concatenate([wq, wk], axis=1)          # (4, 256, D)
        # [d, pair, 256] -> [p, c, pair, 256]
        wqkc = np.ascontiguousarray(
            wqkc.transpose(2, 0, 1).reshape(8, P, 4, 256)
            .transpose(1, 0, 2, 3).astype(bf))
        wvT = np.ascontiguousarray(
            W_qkv[2048 + ds:2048 + ds + 512, :].T
            .reshape(8, P, 512).transpose(1, 0, 2).astype(bf))
        woT = np.ascontiguousarray(
            W_o[:, ds:ds + 512].T.reshape(4, P, D)
            .transpose(1, 0, 2).astype(bf))
        in_maps.append({"xT": xTc, "wqkp": wqkc, "wv": wvT, "wo": woT})
    return in_maps


def kernel(x, W_qkv, W_o):
    from concourse.bass_utils import run_bass_kernel_spmd

    nc = _get_nc()
    in_maps = make_in_maps(np.asarray(x, dtype=np.float32),
                           np.asarray(W_qkv, dtype=np.float32),
                           np.asarray(W_o, dtype=np.float32))
    trace = os.environ.get("KERNEL_TRACE", "") == "1"
    res = run_bass_kernel_spmd(nc, in_maps, core_ids=list(range(NCORES)),
                               trace=trace)
    _cache["last_result"] = res
    Y = np.empty((B, S, D), np.float32)
    for b in range(B):
        Y[b] = res.results[2 * b]["y"] + res.results[2 * b + 1]["y"]
    return Y



# revision 42
# speedup vs baseline: 1.0145x; 1.0145x over previous
"""Trainium2 Bass kernel for CustomMHA (B=4, S=2048, D=1024, H=16).

Sharding: 8 cores = 4 batches x 2 head-groups. Core c handles batch c//2,
heads (c%2)*8 .. (c%2)*8+7. Each core computes its heads' QKV projection,
attention, and a partial output projection (its heads' columns of W_o);
the host sums the two partial Y's per batch.

Per-core structure (bf16 matmuls, fp32 PSUM accumulation):
  - x^T [1024, 2048] resident in SBUF; Q^T/K^T per head-pair as
    [dout, token] tiles (two heads on partition halves 0-63 / 64-127),
    V as [token, head, dh+1] with a ones column for the denominator.
  - scores S^T[k, q] per 128-k tile; the two heads of a pair are packed
    into PE row groups (dh=64 contraction at partition base 0 and 64)
    writing the two halves of one [128, 1024] PSUM tile.
  - softmax: exp on ScalarE with 1/sqrt(d_h) folded into the activation
    scale; no max-subtraction (|scores|/8 stays < ~5).
  - AV: lhsT = [V_h | 1] (M=65), so PSUM row 64 accumulates the softmax
    denominator for free. AV matmuls trail the exp by 2 k-tiles so their
    LDWEIGHTS is never gated on the exp semaphore.
  - normalization: the [65,512] PSUM->SBUF copies run on ScalarE (off the
    DVE FIFO, so the next qb's AV-start isn't gated behind the chain),
    then DVE den-row bounce to partition 0 + reciprocal_approx_fast
    (the custom DVE op needs partition-aligned in/out) + gpsimd
    partition_broadcast + DVE multiply. Head b bounces through a
    [64,512] tile + SBUF->SBUF DMA to reach partitions 64-127.
  - projection: Y[token, e] accumulated over the 4 pair-chunks.
Schedule: x streams in two 1024-column DMA waves; the prologue emits
K0/Q0-tb0/V0 per wave so the PE starts ~6us after the preamble. The
rest of Q0, pairs 1-3 V (N=384 pass) and each next pair's QK interleave
into the exp-bound attention ktile loop (consume 2-3 generator steps
per ktile). Output projection for q-block g interleaves into pair-3's
qb g+1; 28 scratch matmuls keep the PE HAM clock-gate warm through the
final norm chain so the tail projection runs at 2.4GHz.

fp8 (DoubleRow) variants for AV/QKV/proj were numerically simulated and
rejected: e4m3 rounding of pt/V alone gives rel_err ~0.023 > the 2e-2
gate (see precision_sim2.py). The kernel is PE-bound at bf16's
1 column/cycle; measured PE busy ~350us of ~396us span.
"""

import os
import numpy as np
import ml_dtypes

B, S, D, H, DH = 4, 2048, 1024, 16, 64
NCORES = 8
P = 128

_cache = {}


def _build():
    import concourse.bacc as bacc
    import concourse.tile as tile
    from concourse import mybir

    f32 = mybir.dt.float32
    bf16 = mybir.dt.bfloat16
    Exp = mybir.ActivationFunctionType.Exp

    nc = bacc.Bacc("TRN2", target_bir_lowering=False, debug=False)
    xT = nc.dram_tensor("xT", [P, 8, S], bf16, kind="ExternalInput")
    # wqkp: [d, pair, 256] pair-major (cols 0-127 Q-dout, 128-255 K-dout)
    wqkp = nc.dram_tensor("wqkp", [P, 8, 4, 256], bf16, kind="ExternalInput")
    wv = nc.dram_tensor("wv", [P, 8, 512], bf16, kind="ExternalInput")
    wo = nc.dram_tensor("wo", [P, 4, D], bf16, kind="ExternalInput")
    y = nc.dram_tensor("y", [S, D], f32, kind="ExternalOutput")

    with tile.TileContext(nc) as tc:
        import contextlib
        stack = contextlib.ExitStack()
        with stack:
            sb = stack.enter_context(tc.tile_pool(name="sb", bufs=1))
            ptp = stack.enter_context(tc.tile_pool(name="ptp", bufs=18))
            nrm = stack.enter_context(tc.tile_pool(name="nrm", bufs=2))
            otbp = stack.enter_context(tc.tile_pool(name="otb", bufs=4))
            yp = stack.enter_context(tc.tile_pool(name="yp", bufs=2))
            # PSUM: scores 2x[128,1024] (8KB) + AV 2x[65,512] (4KB) +
            # qkv 2x[128,512] (4KB, reused by proj after close) = 16KB
            psS = stack.enter_context(tc.tile_pool(name="psS", bufs=2, space="PSUM"))
            psO = stack.enter_context(tc.tile_pool(name="psO", bufs=1, space="PSUM"))

            qts = [sb.tile([P, S], bf16, tag=f"qt{p}", name=f"qt{p}") for p in range(4)]
            kts = [sb.tile([P, S], bf16, tag=f"kt{p}", name=f"kt{p}") for p in range(4)]
            ots = [[sb.tile([P, 512], bf16, tag=f"ot{p}_{q}", name=f"ot{p}_{q}")
                    for q in range(4)] for p in range(4)]
            vts = [sb.tile([P, 16, 2, 65], bf16, tag=f"vt{p}", name=f"vt{p}") for p in range(4)]
            wo_sb = sb.tile([P, 4, D], bf16)
            x_sbs = [sb.tile([P, S], bf16, tag=f"x{c}", name=f"x{c}")
                     for c in range(8)]
            wqk_sbs = [sb.tile([P, 8, 256], bf16, tag=f"wqk{j}", name=f"wqk{j}")
                       for j in range(4)]
            wv_sb = sb.tile([P, 8, 512], bf16)

            # input DMAs (all partition-major contiguous). x streams in two
            # 1024-column waves (2KB per partition row keeps DMA descriptors
            # efficient) so the prologue's K0/Q0/V0 matmuls start after the
            # first wave instead of waiting for the full 4MB.
            nc.sync.dma_start(out=wqk_sbs[0][:], in_=wqkp[:, :, 0, :])
            for w in range(2):
                for c in range(8):
                    if w == 0 and c < 2:
                        # first chunks split across partition halves (two DMA
                        # queues) so the prologue's first matmuls start sooner
                        for h in (0, 1):
                            nc.sync.dma_start(
                                out=x_sbs[c][64 * h:64 * (h + 1), 0:1024],
                                in_=xT[64 * h:64 * (h + 1), c, 0:1024])
                    else:
                        nc.sync.dma_start(
                            out=x_sbs[c][:, w * 1024:(w + 1) * 1024],
                            in_=xT[:, c, w * 1024:(w + 1) * 1024])
                if w == 0:
                    nc.sync.dma_start(out=wv_sb[:], in_=wv[:])
            for j in range(1, 4):
                nc.sync.dma_start(out=wqk_sbs[j][:], in_=wqkp[:, :, j, :])
            nc.sync.dma_start(out=wo_sb[:], in_=wo[:])
            for p in range(4):
                nc.vector.memset(vts[p][:, :, :, 64:65], 1.0)

            def gen_qk(hp, pool, halves=(0, 1), tbs=(0, 1, 2, 3)):
                for half in halves:
                    dst = qts[hp] if half == 0 else kts[hp]
                    for tb in tbs:
                        ps = pool.tile([P, 512], f32, tag="ps", name="ps")
                        for c in range(8):
                            nc.tensor.matmul(
                                ps[:],
                                lhsT=wqk_sbs[hp][:, c, half * 128:(half + 1) * 128],
                                rhs=x_sbs[c][:, tb * 512:(tb + 1) * 512],
                                start=(c == 0), stop=(c == 7),
                            )
                            if c == 7:
                                nc.vector.tensor_copy(
                                    dst[:, tb * 512:(tb + 1) * 512], ps[:])
                            yield

            def gen_v(pool, pairs, ts=tuple(range(16))):
                # V projection for a contiguous run of head-pairs. Pair 0 runs
                # serially in the prologue (qb0's AV needs all 16 k-chunks);
                # pairs 1-3 interleave into pair-0/1 attention.
                lo, hi = pairs[0] * 128, (pairs[-1] + 1) * 128
                for t in ts:
                    ps = pool.tile([P, 512], f32, tag="ps", name="ps")
                    for c in range(8):
                        nc.tensor.matmul(
                            ps[:, 0:hi - lo],
                            lhsT=x_sbs[c][:, t * 128:(t + 1) * 128],
                            rhs=wv_sb[:, c, lo:hi],
                            start=(c == 0), stop=(c == 7),
                        )
                        if c == 7:
                            for k in pairs:
                                nc.vector.tensor_copy(
                                    vts[k][:, t, :, 0:64],
                                    ps[:, k * 128 - lo:(k + 1) * 128 - lo].rearrange(
                                        "p (h d) -> p h d", d=64))
                        yield

            def gen_proj_qb(g, pool):
                # projection for token tiles of q-block g (needs all ots[*][g]).
                # The no-op prefix delays the first matmul past the norm+bounce
                # chain that produces ots[3][g].
                for _ in range(16):
                    yield
                for tq in range(4):
                    t = g * 4 + tq
                    for eh in range(2):
                        ps = pool.tile([P, 512], f32, tag="psy", name="ps")
                        for c in range(4):
                            nc.tensor.matmul(
                                ps[:],
                                lhsT=ots[c][g][:, tq * 128:(tq + 1) * 128],
                                rhs=wo_sb[:, c, eh * 512:(eh + 1) * 512],
                                start=(c == 0), stop=(c == 3),
                            )
                            if c == 3:
                                ysb = yp.tile([P, 512], f32, tag="ysb", name="ysb")
                                nc.vector.tensor_copy(ysb[:], ps[:])
                                nc.sync.dma_start(
                                    out=y[t * 128:(t + 1) * 128,
                                          eh * 512:(eh + 1) * 512],
                                    in_=ysb[:])
                            yield

            work = []

            def consume(n):
                for _ in range(n):
                    while work:
                        try:
                            next(work[0])
                            break
                        except StopIteration:
                            work.pop(0)
                    else:
                        break

            def drain_work():
                while work:
                    for _ in work.pop(0):
                        pass

            def emit_attn(hp, after_qb=None, rate0=3):
                qt, kt, vt = qts[hp], kts[hp], vts[hp]
                for qb in range(4):
                    qsl = slice(qb * 512, (qb + 1) * 512)
                    ot = ots[hp][qb]
                    oa = psO.tile([65, 512], f32, tag="oa")
                    ob = psO.tile([65, 512], f32, tag="ob")
                    pts = [None] * 16

                    def emit_av(kti):
                        # on the stop tile, finish ob first: the osb copy
                        # (head b, which gates the next qb's AV-start and
                        # carries the bounce DMA) can then begin ~400ns sooner
                        heads = (1, 0) if kti == 15 else (0, 1)
                        for h in heads:
                            nc.tensor.matmul(
                                (oa, ob)[h][:], lhsT=vt[:, kti, h, :],
                                rhs=pts[kti][:, h * 512:(h + 1) * 512],
                                start=(kti == 0), stop=(kti == 15))

                    for kti in range(16):
                        ksl = slice(kti * 128, (kti + 1) * 128)
                        if kti >= 2:
                            emit_av(kti - 2)
                        s = psS.tile([P, 1024], f32, tag="s")
                        nc.tensor.matmul(
                            s[:, 0:512], lhsT=kt[0:64, ksl], rhs=qt[0:64, qsl],
                            start=True, stop=True)
                        nc.tensor.matmul(
                            s[:, 512:1024], lhsT=kt[64:128, ksl], rhs=qt[64:128, qsl],
                            start=True, stop=True)
                        pt = ptp.tile([P, 1024], bf16, tag="pt")
                        pts[kti] = pt
                        nc.scalar.activation(pt[:], s[:], Exp, scale=0.125)
                        consume(rate0 if qb == 0 else 3)
                    for kti in (14, 15):
                        emit_av(kti)
                    # free the AV psum banks fast: copy to SBUF on ScalarE
                    # (keeps the copies off the DVE FIFO so the next qb's
                    # AV-start isn't gated on the norm chain), then normalize.
                    # Head b first: its path carries the extra SBUF->SBUF
                    # bounce DMA, so it gates downstream readers.
                    osb = nrm.tile([65, 512], f32, tag="osb")
                    nc.scalar.copy(osb[:], ob[:])
                    osa = nrm.tile([65, 512], f32, tag="osa")
                    nc.scalar.copy(osa[:], oa[:])
                    # reciprocal_approx_fast needs partition-aligned in/out:
                    # bounce the denominator row from partition 64 to 0 first.
                    dnb = nrm.tile([1, 512], f32, tag="dnb")
                    nc.vector.tensor_copy(dnb[:], osb[64:65, :])
                    rcb = nrm.tile([1, 512], f32, tag="rcb")
                    nc.vector.reciprocal_approx_fast(rcb[:], dnb[:])
                    bcb = nrm.tile([64, 512], f32, tag="bcb")
                    nc.gpsimd.partition_broadcast(bcb[:], rcb[:])
                    otb = otbp.tile([64, 512], bf16, tag="otb")
                    nc.vector.tensor_mul(otb[:], osb[0:64, :], bcb[:])
                    nc.sync.dma_start(out=ot[64:128, :], in_=otb[:])
                    # head a (lanes aligned 0-63)
                    dna = nrm.tile([1, 512], f32, tag="dna")
                    nc.vector.tensor_copy(dna[:], osa[64:65, :])
                    rca = nrm.tile([1, 512], f32, tag="rca")
                    nc.vector.reciprocal_approx_fast(rca[:], dna[:])
                    bca = nrm.tile([64, 512], f32, tag="bca")
                    nc.gpsimd.partition_broadcast(bca[:], rca[:])
                    nc.vector.tensor_mul(ot[0:64, :], osa[0:64, :], bca[:])
                    if after_qb is not None:
                        after_qb(qb)

            with tc.tile_pool(name="psA", bufs=2, space="PSUM") as psA:
                # serial prologue, paced by the x DMA waves: K0 + Q0-tb0 +
                # pair-0 V, each wave's work emitted as its x columns land.
                # The rest of Q0, pairs 1-3 V, and pair-1 QK interleave into
                # attention.
                for w in range(2):
                    for _ in gen_qk(0, psA, halves=(1,), tbs=(2 * w, 2 * w + 1)):
                        pass
                    if w == 0:
                        for _ in gen_qk(0, psA, halves=(0,), tbs=(0,)):
                            pass
                    for _ in gen_v(psA, (0,), ts=tuple(range(8 * w, 8 * w + 8))):
                        pass
                work[:] = [gen_qk(0, psA, halves=(0,), tbs=(1, 2, 3)),
                           gen_qk(1, psA),
                           gen_v(psA, (1, 2, 3))]
                emit_attn(0, rate0=2)
                work.append(gen_qk(2, psA))
                emit_attn(1)
                drain_work()
                work[:] = [gen_qk(3, psA)]
                emit_attn(2)
                drain_work()

            # ---- pair 3 attention + interleaved output projection ----
            with tc.tile_pool(name="psC", bufs=2, space="PSUM") as psC:
                def after3(qb):
                    work.append(gen_proj_qb(qb, psC))
                    if qb == 3:
                        # scratch matmuls keep the PE (HAM clock gate) warm
                        # through the final norm chain so the tail projection
                        # runs at 2.4GHz instead of re-throttled 1.2GHz
                        for _ in range(28):
                            sw = psS.tile([P, 1024], f32, tag="s", name="sw")
                            nc.tensor.matmul(
                                sw[:, 0:512], lhsT=kts[3][0:64, 0:128],
                                rhs=qts[3][0:64, 0:512], start=True, stop=True)

                emit_attn(3, after_qb=after3)
                drain_work()

    nc.compile()
    return nc


def _get_nc():
    if "nc" not in _cache:
        _cache["nc"] = _build()
    return _cache["nc"]


def make_in_maps(x, W_qkv, W_o):
    # All tensors pre-arranged partition-major [128, ...] so each input DMA
    # is 128 large contiguous descriptors.
    bf = ml_dtypes.bfloat16
    in_maps = []
    for c in range(NCORES):
        b, g = c // 2, c % 2
        ds = g * 512  # this core's slice of the head-major model dim
        # x^T [d, s] -> [p, c, s]
        xTc = np.ascontiguousarray(
            x[b].T.reshape(8, P, S).transpose(1, 0, 2).astype(bf))
        wq = W_qkv[ds:ds + 512, :].reshape(4, P, D)
        wk = W_qkv[1024 + ds:1024 + ds + 512, :].reshape(4, P, D)
        wqkc = np.concatenate([wq, wk], axis=1)          # (4, 256, D)
        # [d, pair, 256] -> [p, c, pair, 256]
        wqkc = np.ascontiguousarray(
            wqkc.transpose(2, 0, 1).reshape(8, P, 4, 256)
            .transpose(1, 0, 2, 3).astype(bf))
        wvT = np.ascontiguousarray(
            W_qkv[2048 + ds:2048 + ds + 512, :].T
            .reshape(8, P, 512).transpose(1, 0, 2).astype(bf))
        woT = np.ascontiguousarray(
            W_o[:, ds:ds + 512].T.reshape(4, P, D)
            .transpose(1, 0, 2).astype(bf))
        in_maps.append({"xT": xTc, "wqkp": wqkc, "wv": wvT, "wo": woT})
    return in_maps


def kernel(x, W_qkv, W_o):
    from concourse.bass_utils import run_bass_kernel_spmd

    nc = _get_nc()
    in_maps = make_in_maps(np.asarray(x, dtype=np.float32),
                           np.asarray(W_qkv, dtype=np.float32),
                           np.asarray(W_o, dtype=np.float32))
    trace = os.environ.get("KERNEL_TRACE", "") == "1"
    res = run_bass_kernel_spmd(nc, in_maps, core_ids=list(range(NCORES)),
                               trace=trace)
    _cache["last_result"] = res
    Y = np.empty((B, S, D), np.float32)
    for b in range(B):
        Y[b] = res.results[2 * b]["y"] + res.results[2 * b + 1]["y"]
    return Y



# revision 44
# speedup vs baseline: 1.0288x; 1.0141x over previous
"""Trainium2 Bass kernel for CustomMHA (B=4, S=2048, D=1024, H=16).

Sharding: 8 cores = 4 batches x 2 head-groups. Core c handles batch c//2,
heads (c%2)*8 .. (c%2)*8+7. Each core computes its heads' QKV projection,
attention, and a partial output projection (its heads' columns of W_o);
the host sums the two partial Y's per batch.

Per-core structure (bf16 matmuls, fp32 PSUM accumulation):
  - x^T [1024, 2048] resident in SBUF; Q^T/K^T per head-pair as
    [dout, token] tiles (two heads on partition halves 0-63 / 64-127),
    V as [token, head, dh+1] with a ones column for the denominator.
  - scores S^T[k, q] per 128-k tile; the two heads of a pair are packed
    into PE row groups (dh=64 contraction at partition base 0 and 64)
    writing the two halves of one [128, 1024] PSUM tile.
  - softmax: exp on ScalarE with 1/sqrt(d_h) folded into the activation
    scale; no max-subtraction (|scores|/8 stays < ~5).
  - AV: lhsT = [V_h | 1] (M=65), so PSUM row 64 accumulates the softmax
    denominator for free. AV matmuls trail the exp by 2 k-tiles so their
    LDWEIGHTS is never gated on the exp semaphore.
  - normalization: the [65,512] PSUM->SBUF copies run on ScalarE (off the
    DVE FIFO, so the next qb's AV-start isn't gated behind the chain),
    then DVE den-row bounce to partition 0 + reciprocal_approx_fast
    (the custom DVE op needs partition-aligned in/out) + gpsimd
    partition_broadcast + DVE multiply. Head b bounces through a
    [64,512] tile + SBUF->SBUF DMA to reach partitions 64-127.
  - projection: Y[token, e] accumulated over the 4 pair-chunks.
Schedule: x streams in two 1024-column DMA waves; the prologue emits
K0/Q0-tb0/V0 per wave so the PE starts ~6us after the preamble. The
rest of Q0, pairs 1-3 V (N=384 pass) and each next pair's QK interleave
into the exp-bound attention ktile loop (consume 2-3 generator steps
per ktile). Output projection for q-block g interleaves into pair-3's
qb g+1; 28 scratch matmuls keep the PE HAM clock-gate warm through the
final norm chain so the tail projection runs at 2.4GHz.

fp8 (DoubleRow) variants for AV/QKV/proj were numerically simulated and
rejected: e4m3 rounding of pt/V alone gives rel_err ~0.023 > the 2e-2
gate (see precision_sim2.py). The kernel is PE-bound at bf16's
1 column/cycle; measured PE busy ~350us of ~396us span.
"""

import os
import numpy as np
import ml_dtypes

B, S, D, H, DH = 4, 2048, 1024, 16, 64
NCORES = 8
P = 128

_cache = {}


def _build():
    import concourse.bacc as bacc
    import concourse.tile as tile
    from concourse import mybir

    f32 = mybir.dt.float32
    bf16 = mybir.dt.bfloat16
    Exp = mybir.ActivationFunctionType.Exp

    nc = bacc.Bacc("TRN2", target_bir_lowering=False, debug=False)
    xT = nc.dram_tensor("xT", [P, 8, S], bf16, kind="ExternalInput")
    # wqkp: [d, pair, 256] pair-major (cols 0-127 Q-dout, 128-255 K-dout)
    wqkp = nc.dram_tensor("wqkp", [P, 8, 4, 256], bf16, kind="ExternalInput")
    wv = nc.dram_tensor("wv", [P, 8, 512], bf16, kind="ExternalInput")
    wo = nc.dram_tensor("wo", [P, 4, D], bf16, kind="ExternalInput")
    y = nc.dram_tensor("y", [S, D], f32, kind="ExternalOutput")

    with tile.TileContext(nc) as tc:
        import contextlib
        stack = contextlib.ExitStack()
        with stack:
            sb = stack.enter_context(tc.tile_pool(name="sb", bufs=1))
            ptp = stack.enter_context(tc.tile_pool(name="ptp", bufs=18))
            nrm = stack.enter_context(tc.tile_pool(name="nrm", bufs=2))
            otbp = stack.enter_context(tc.tile_pool(name="otb", bufs=4))
            yp = stack.enter_context(tc.tile_pool(name="yp", bufs=3))
            # PSUM: scores 2x[128,1024] (8KB) + AV 2x[65,512] (4KB) +
            # qkv 2x[128,512] (4KB, reused by proj after close) = 16KB
            psS = stack.enter_context(tc.tile_pool(name="psS", bufs=2, space="PSUM"))
            psO = stack.enter_context(tc.tile_pool(name="psO", bufs=1, space="PSUM"))

            qts = [sb.tile([P, S], bf16, tag=f"qt{p}", name=f"qt{p}") for p in range(4)]
            kts = [sb.tile([P, S], bf16, tag=f"kt{p}", name=f"kt{p}") for p in range(4)]
            ots = [[sb.tile([P, 512], bf16, tag=f"ot{p}_{q}", name=f"ot{p}_{q}")
                    for q in range(4)] for p in range(4)]
            vts = [sb.tile([P, 16, 2, 65], bf16, tag=f"vt{p}", name=f"vt{p}") for p in range(4)]
            wo_sb = sb.tile([P, 4, D], bf16)
            x_sbs = [sb.tile([P, S], bf16, tag=f"x{c}", name=f"x{c}")
                     for c in range(8)]
            wqk_sbs = [sb.tile([P, 8, 256], bf16, tag=f"wqk{j}", name=f"wqk{j}")
                       for j in range(4)]
            wv_sb = sb.tile([P, 8, 512], bf16)

            # input DMAs (all partition-major contiguous). x streams in two
            # 1024-column waves (2KB per partition row keeps DMA descriptors
            # efficient) so the prologue's K0/Q0/V0 matmuls start after the
            # first wave instead of waiting for the full 4MB.
            nc.sync.dma_start(out=wqk_sbs[0][:], in_=wqkp[:, :, 0, :])
            for w in range(2):
                for c in range(8):
                    if w == 0 and c < 2:
                        # first chunks split across partition halves (two DMA
                        # queues) so the prologue's first matmuls start sooner
                        for h in (0, 1):
                            nc.sync.dma_start(
                                out=x_sbs[c][64 * h:64 * (h + 1), 0:1024],
                                in_=xT[64 * h:64 * (h + 1), c, 0:1024])
                    else:
                        nc.sync.dma_start(
                            out=x_sbs[c][:, w * 1024:(w + 1) * 1024],
                            in_=xT[:, c, w * 1024:(w + 1) * 1024])
                if w == 0:
                    nc.sync.dma_start(out=wv_sb[:], in_=wv[:])
            for j in range(1, 4):
                nc.sync.dma_start(out=wqk_sbs[j][:], in_=wqkp[:, :, j, :])
            nc.sync.dma_start(out=wo_sb[:], in_=wo[:])
            for p in range(4):
                nc.vector.memset(vts[p][:, :, :, 64:65], 1.0)

            def gen_qk(hp, pool, halves=(0, 1), tbs=(0, 1, 2, 3)):
                for half in halves:
                    dst = qts[hp] if half == 0 else kts[hp]
                    for tb in tbs:
                        ps = pool.tile([P, 512], f32, tag="ps", name="ps")
                        for c in range(8):
                            nc.tensor.matmul(
                                ps[:],
                                lhsT=wqk_sbs[hp][:, c, half * 128:(half + 1) * 128],
                                rhs=x_sbs[c][:, tb * 512:(tb + 1) * 512],
                                start=(c == 0), stop=(c == 7),
                            )
                            if c == 7:
                                nc.vector.tensor_copy(
                                    dst[:, tb * 512:(tb + 1) * 512], ps[:])
                            yield

            def gen_v(pool, pairs, ts=tuple(range(16))):
                # V projection for a contiguous run of head-pairs. Pair 0 runs
                # serially in the prologue (qb0's AV needs all 16 k-chunks);
                # pairs 1-3 interleave into pair-0/1 attention.
                lo, hi = pairs[0] * 128, (pairs[-1] + 1) * 128
                for t in ts:
                    ps = pool.tile([P, 512], f32, tag="ps", name="ps")
                    for c in range(8):
                        nc.tensor.matmul(
                            ps[:, 0:hi - lo],
                            lhsT=x_sbs[c][:, t * 128:(t + 1) * 128],
                            rhs=wv_sb[:, c, lo:hi],
                            start=(c == 0), stop=(c == 7),
                        )
                        if c == 7:
                            for k in pairs:
                                nc.vector.tensor_copy(
                                    vts[k][:, t, :, 0:64],
                                    ps[:, k * 128 - lo:(k + 1) * 128 - lo].rearrange(
                                        "p (h d) -> p h d", d=64))
                        yield

            def gen_proj_qb(g, pool):
                # projection for token tiles of q-block g (needs all ots[*][g]).
                # The no-op prefix delays the first matmul past the norm+bounce
                # chain that produces ots[3][g].
                for _ in range(16):
                    yield
                for tq in range(4):
                    t = g * 4 + tq
                    for eh in range(2):
                        ps = pool.tile([P, 512], f32, tag="psy", name="ps")
                        for c in range(4):
                            nc.tensor.matmul(
                                ps[:],
                                lhsT=ots[c][g][:, tq * 128:(tq + 1) * 128],
                                rhs=wo_sb[:, c, eh * 512:(eh + 1) * 512],
                                start=(c == 0), stop=(c == 3),
                            )
                            if c == 3:
                                ysb = yp.tile([P, 512], f32, tag="ysb", name="ysb")
                                nc.vector.tensor_copy(ysb[:], ps[:])
                                nc.sync.dma_start(
                                    out=y[t * 128:(t + 1) * 128,
                                          eh * 512:(eh + 1) * 512],
                                    in_=ysb[:])
                            yield

            work = []

            def consume(n):
                for _ in range(n):
                    while work:
                        try:
                            next(work[0])
                            break
                        except StopIteration:
                            work.pop(0)
                    else:
                        break

            def drain_work():
                while work:
                    for _ in work.pop(0):
                        pass

            def emit_attn(hp, after_qb=None, rate0=3):
                qt, kt, vt = qts[hp], kts[hp], vts[hp]
                for qb in range(4):
                    qsl = slice(qb * 512, (qb + 1) * 512)
                    ot = ots[hp][qb]
                    oa = psO.tile([65, 512], f32, tag="oa")
                    ob = psO.tile([65, 512], f32, tag="ob")
                    pts = [None] * 16

                    def emit_av(kti):
                        # on the stop tile, finish ob first: the osb copy
                        # (head b, which gates the next qb's AV-start and
                        # carries the bounce DMA) can then begin ~400ns sooner
                        heads = (1, 0) if kti == 15 else (0, 1)
                        for h in heads:
                            nc.tensor.matmul(
                                (oa, ob)[h][:], lhsT=vt[:, kti, h, :],
                                rhs=pts[kti][:, h * 512:(h + 1) * 512],
                                start=(kti == 0), stop=(kti == 15))

                    for kti in range(16):
                        ksl = slice(kti * 128, (kti + 1) * 128)
                        if kti >= 2:
                            emit_av(kti - 2)
                        s = psS.tile([P, 1024], f32, tag="s")
                        nc.tensor.matmul(
                            s[:, 0:512], lhsT=kt[0:64, ksl], rhs=qt[0:64, qsl],
                            start=True, stop=True)
                        nc.tensor.matmul(
                            s[:, 512:1024], lhsT=kt[64:128, ksl], rhs=qt[64:128, qsl],
                            start=True, stop=True)
                        pt = ptp.tile([P, 1024], bf16, tag="pt")
                        pts[kti] = pt
                        nc.scalar.activation(pt[:], s[:], Exp, scale=0.125)
                        consume(rate0 if qb == 0 else 3)
                    for kti in (14, 15):
                        emit_av(kti)
                    # free the AV psum banks fast: copy to SBUF on ScalarE
                    # (keeps the copies off the DVE FIFO so the next qb's
                    # AV-start isn't gated on the norm chain), then normalize.
                    # Head b first: its path carries the extra SBUF->SBUF
                    # bounce DMA, so it gates downstream readers.
                    osb = nrm.tile([65, 512], f32, tag="osb")
                    nc.scalar.copy(osb[:], ob[:])
                    osa = nrm.tile([65, 512], f32, tag="osa")
                    nc.scalar.copy(osa[:], oa[:])
                    # reciprocal_approx_fast needs partition-aligned in/out:
                    # bounce the denominator row from partition 64 to 0 first.
                    dnb = nrm.tile([1, 512], f32, tag="dnb")
                    nc.vector.tensor_copy(dnb[:], osb[64:65, :])
                    rcb = nrm.tile([1, 512], f32, tag="rcb")
                    nc.vector.reciprocal_approx_fast(rcb[:], dnb[:])
                    bcb = nrm.tile([64, 512], f32, tag="bcb")
                    nc.gpsimd.partition_broadcast(bcb[:], rcb[:])
                    otb = otbp.tile([64, 512], bf16, tag="otb")
                    nc.vector.tensor_mul(otb[:], osb[0:64, :], bcb[:])
                    nc.sync.dma_start(out=ot[64:128, :], in_=otb[:])
                    # head a (lanes aligned 0-63)
                    dna = nrm.tile([1, 512], f32, tag="dna")
                    nc.vector.tensor_copy(dna[:], osa[64:65, :])
                    rca = nrm.tile([1, 512], f32, tag="rca")
                    nc.vector.reciprocal_approx_fast(rca[:], dna[:])
                    bca = nrm.tile([64, 512], f32, tag="bca")
                    nc.gpsimd.partition_broadcast(bca[:], rca[:])
                    nc.vector.tensor_mul(ot[0:64, :], osa[0:64, :], bca[:])
                    if after_qb is not None:
                        after_qb(qb)

            with tc.tile_pool(name="psA", bufs=2, space="PSUM") as psA:
                # serial prologue, paced by the x DMA waves: K0 + Q0-tb0 +
                # pair-0 V, each wave's work emitted as its x columns land.
                # The rest of Q0, pairs 1-3 V, and pair-1 QK interleave into
                # attention.
                for w in range(2):
                    for _ in gen_qk(0, psA, halves=(1,), tbs=(2 * w, 2 * w + 1)):
                        pass
                    if w == 0:
                        for _ in gen_qk(0, psA, halves=(0,), tbs=(0,)):
                            pass
                    for _ in gen_v(psA, (0,), ts=tuple(range(8 * w, 8 * w + 8))):
                        pass
                work[:] = [gen_qk(0, psA, halves=(0,), tbs=(1, 2, 3)),
                           gen_qk(1, psA),
                           gen_v(psA, (1, 2, 3))]
                emit_attn(0, rate0=2)
                work.append(gen_qk(2, psA))
                emit_attn(1)
                drain_work()
                work[:] = [gen_qk(3, psA)]
                emit_attn(2)
                drain_work()

            # ---- pair 3 attention + interleaved output projection ----
            with tc.tile_pool(name="psC", bufs=2, space="PSUM") as psC:
                def after3(qb):
                    work.append(gen_proj_qb(qb, psC))
                    if qb == 3:
                        # scratch matmuls keep the PE (HAM clock gate) warm
                        # through the final norm chain so the tail projection
                        # runs at 2.4GHz instead of re-throttled 1.2GHz
                        for _ in range(28):
                            sw = psS.tile([P, 1024], f32, tag="s", name="sw")
                            nc.tensor.matmul(
                                sw[:, 0:512], lhsT=kts[3][0:64, 0:128],
                                rhs=qts[3][0:64, 0:512], start=True, stop=True)

                emit_attn(3, after_qb=after3)
                drain_work()

    nc.compile()
    return nc


def _get_nc():
    if "nc" not in _cache:
        _cache["nc"] = _build()
    return _cache["nc"]


def make_in_maps(x, W_qkv, W_o):
    # All tensors pre-arranged partition-major [128, ...] so each input DMA
    # is 128 large contiguous descriptors.
    bf = ml_dtypes.bfloat16
    in_maps = []
    for c in range(NCORES):
        b, g = c // 2, c % 2
        ds = g * 512  # this core's slice of the head-major model dim
        # x^T [d, s] -> [p, c, s]
        xTc = np.ascontiguousarray(
            x[b].T.reshape(8, P, S).transpose(1, 0, 2).astype(bf))
        wq = W_qkv[ds:ds + 512, :].reshape(4, P, D)
        wk = W_qkv[1024 + ds:1024 + ds + 512, :].reshape(4, P, D)
        wqkc = np.concatenate([wq, wk], axis=1)          # (4, 256, D)
        # [d, pair, 256] -> [p, c, pair, 256]
        wqkc = np.ascontiguousarray(
            wqkc.transpose(2, 0, 1).reshape(8, P, 4, 256)
            .transpose(1, 0, 2, 3).astype(bf))
        wvT = np.ascontiguousarray(
            W_qkv[2048 + ds:2048 + ds + 512, :].T
            .reshape(8, P, 512).transpose(1, 0, 2).astype(bf))
        woT = np.ascontiguousarray(
            W_o[:, ds:ds + 512].T.reshape(4, P, D)
            .transpose(1, 0, 2).astype(bf))
        in_maps.append({"xT": xTc, "wqkp": wqkc, "wv": wvT, "wo": woT})
    return in_maps


def kernel(x, W_qkv, W_o):
    from concourse.bass_utils import run_bass_kernel_spmd

    nc = _get_nc()
    in_maps = make_in_maps(np.asarray(x, dtype=np.float32),
                           np.asarray(W_qkv, dtype=np.float32),
                           np.asarray(W_o, dtype=np.float32))
    trace = os.environ.get("KERNEL_TRACE", "") == "1"
    res = run_bass_kernel_spmd(nc, in_maps, core_ids=list(range(NCORES)),
                               trace=trace)
    _cache["last_result"] = res
    Y = np.empty((B, S, D), np.float32)
    for b in range(B):
        Y[b] = res.results[2 * b]["y"] + res.results[2 * b + 1]["y"]
    return Y

